# revision 2
# baseline (speedup 1.0000x reference)
"""Trainium2 Bass kernel v2 for the DPPNMT seq2seq LSTM+attention model.

Sharding: data-parallel over batch (64 -> 8 per core), params replicated.

v2 redesign vs baseline:
- All-tanh LSTM cells: sigmoid gates computed as tanh(x/2) with weight rows
  pre-halved on the host; cell math uses fused scalar_tensor_tensor ops with
  h and c states stored DOUBLED (H2=2h, C2=2c), scale factors folded into
  weights.  The whole recurrence + attention exp then lives in one
  activation table set (exp_and_others: Exp/Tanh/Identity/Copy) -> zero
  LoadActFuncSet reloads (baseline paid 128 x 1.28us).
- One fused gate activation per step (gate rows reordered to i,f,o,g).
- fwd+bwd encoder directions fused into shared [128,128] ops.
- zx/zy added into the z PSUM via one identity matmul (no DVE zsum add).
- Attention: pair-batched e matmuls (8 instead of 16), exp written straight
  into a persistent block-diagonal tile, softmax denominator broadcast via a
  ones-matrix matmul ([128,8] Z in one shot), Wcomb_a folded into a
  precomputed encW = 0.5*Wcomb_a @ enc_hiddens so the context directly
  accumulates into the output projection PSUM.
"""

from contextlib import ExitStack

import numpy as np
import ml_dtypes

import concourse.bass as bass
import concourse.tile as tile
from concourse import bacc, mybir
from concourse.bass_utils import run_bass_kernel_spmd
from concourse.masks import make_identity

BF16 = mybir.dt.bfloat16
F32 = mybir.dt.float32
AF = mybir.ActivationFunctionType
ALU = mybir.AluOpType

S, T, B, E, H, V = 64, 64, 64, 256, 256, 32000
NCORES = 8
BL = B // NCORES          # local batch = 8
TD = T - 1                # decoder steps = 63
GCH = 8                   # gate chunks (4H/128)
ECH = 2
HCH = 2
NR = TD * BL              # 504 vocab rows per core
VBLK = 2000               # vocab cols per block
NVB = V // VBLK           # 16
HST = 65 * 8              # h history stride per (dir, ch): slots 0..64
OST = (TD + 1) * 8
bf16 = ml_dtypes.bfloat16
INTERLEAVE_VOCAB = False


def build_program():
    nc = bacc.Bacc("TRN2", target_bir_lowering=False, debug=False)

    def din(name, shape, dt=BF16):
        return nc.dram_tensor(name, shape, dt, kind="ExternalInput").ap()

    # activations (per-core): x as (dir, ech, t, b) for encoder
    xe_t = din("xe_t", [128, S * 2 * ECH * BL])
    yt = din("yt", [128, ECH * TD * BL])
    # weights, all packed as lhsT tiles [128, kch*mch*128]
    wih_f = din("wih_f", [128, ECH * GCH * 128])
    wih_b = din("wih_b", [128, ECH * GCH * 128])
    whh_f = din("whh_f", [128, HCH * GCH * 128])
    whh_b = din("whh_b", [128, HCH * GCH * 128])
    benc_f = din("benc_f", [128, GCH], F32)
    benc_b = din("benc_b", [128, GCH], F32)
    wihe = din("wihe", [128, ECH * GCH * 128])
    wiho = din("wiho", [128, HCH * GCH * 128])
    whhd = din("whhd", [128, HCH * GCH * 128])
    bdec = din("bdec", [128, GCH], F32)
    wcomb_a = din("wcomb_a", [128, 4 * HCH * 128])   # 0.5*Wcomb[:, :2H]
    wcomb_h = din("wcomb_h", [128, HCH * HCH * 128])  # 0.5*Wcomb[:, 2H:]
    wh_l = din("wh_l", [128, 4 * HCH * 128])
    wc_l = din("wc_l", [128, 4 * HCH * 128])
    watt_l = din("watt_l", [128, 4 * HCH * 128])      # 0.25*Watt
    wvt = din("wvt", [128, HCH * V])
    wgt = din("wgt", [128, HCH * NR])
    out_lse = nc.dram_tensor("out_lse", [128, 4], F32,
                             kind="ExternalOutput").ap()
    out_gd = nc.dram_tensor("out_gd", [1, 1024], F32,
                            kind="ExternalOutput").ap()



    with tile.TileContext(nc) as tc:
        with ExitStack() as ctx:
            consts = ctx.enter_context(tc.tile_pool(name="consts", bufs=1))
            wsb = ctx.enter_context(tc.tile_pool(name="wsb", bufs=1))
            state = ctx.enter_context(tc.tile_pool(name="state", bufs=1))

            id128 = consts.tile([128, 128], BF16)
            make_identity(nc, id128[:])
            ones_mat = consts.tile([128, 128], BF16)
            nc.vector.memset(ones_mat[:], 1.0)
            ones_col = consts.tile([128, 1], BF16)
            nc.vector.memset(ones_col[:], 1.0)

            def load(ap_dram, dt=BF16):
                t = wsb.tile(list(ap_dram.shape), dt,
                             tag=ap_dram.tensor.name + "_sb")
                nc.sync.dma_start(t[:], ap_dram[:])
                return t

            xe_sb = load(xe_t)
            yt_sb = load(yt)
            wihf_sb, wihb_sb = load(wih_f), load(wih_b)
            whhf_sb, whhb_sb = load(whh_f), load(whh_b)
            bencf_sb, bencb_sb = load(benc_f, F32), load(benc_b, F32)
            wihe_sb, wiho_sb, whhd_sb = load(wihe), load(wiho), load(whhd)
            bdec_sb = load(bdec, F32)
            wca_sb, wch_sb = load(wcomb_a), load(wcomb_h)
            wh_sb, wc_sb, watt_sb = load(wh_l), load(wc_l), load(watt_l)
            wgt_sb = load(wgt)

            # persistent state
            # h_all: cols (dir, ch, slot, b); slot 0 = h_{-1} = 0
            h_all = state.tile([128, 2 * 2 * HST // 8 * 8], BF16)  # [128,2080]
            hv = h_all[:].rearrange("p (d c t b) -> p d c t b",
                                    d=2, c=2, b=BL)
            c2 = state.tile([128, 32], F32)          # (dir, ch, b) doubled c
            nc.vector.memset(c2[:], 0.0)
            # fwd init = slot 0 (writes upward t+1); bwd init = slot 64
            # (writes downward 63-t so slot == original position)
            for ch in range(2):
                nc.vector.memset(h_all[:, ch * HST:ch * HST + 8], 0.0)
                nc.vector.memset(
                    h_all[:, (2 + ch) * HST + 64 * 8:(2 + ch) * HST + 65 * 8],
                    0.0)
            zx = state.tile([128, S * 128], BF16)    # (t, dir, gch, b)
            zyb = state.tile([128, TD * 64], BF16)   # (t, gch, b)
            outsT = state.tile([128, 2 * OST], BF16)  # (ch, slot, b)
            nc.vector.memset(outsT[:, 0:8], 0.0)
            nc.vector.memset(outsT[:, OST:OST + 8], 0.0)
            encprojT = state.tile([128, HCH * BL * S], BF16)  # (ch,pr,u,s)
            encWT = state.tile([128, HCH * BL * S], BF16)     # (mch, s, b)
            encW_cs = state.tile([128, 8 * 128], BF16)  # (pair,mch):[us x h]
            ablk = state.tile([128, 8], BF16)        # block-diag exp(e)
            ablk_n = state.tile([128, 8], BF16)      # normalized alpha
            nc.vector.memset(ablk[:], 0.0)
            nc.vector.memset(ablk_n[:], 0.0)
            se_parts = state.tile([128, 4 * NVB], F32)
            nc.vector.memset(se_parts[:], 1.0)
            lse_sb = state.tile([128, 4], F32)
            gd_sb = state.tile([1, 1024], F32)
            nc.vector.memset(gd_sb[:], 0.0)
            tmp_gd = state.tile([128, 2 * NR], BF16)

            with ExitStack() as rctx:
                pz = rctx.enter_context(
                    tc.tile_pool(name="pz", bufs=2, space="PSUM"))
                psmall = rctx.enter_context(
                    tc.tile_pool(name="psmall", bufs=2, space="PSUM"))
                work = rctx.enter_context(tc.tile_pool(name="work", bufs=2))
                gctx = ExitStack()
                pep = gctx.enter_context(
                    tc.tile_pool(name="pep", bufs=1, space="PSUM"))

                # ---- phase 1: zx = x @ Wih^T + b for enc dirs; zy dec ----
                zxv = zx[:].rearrange("p (t d g b) -> p t d g b",
                                      d=2, g=GCH, b=BL)
                xev = xe_sb[:].rearrange("p (d e t b) -> p d e t b",
                                         d=2, e=ECH, b=BL)
                for d, (wih_sb, b_sb) in enumerate(
                        ((wihf_sb, bencf_sb), (wihb_sb, bencb_sb))):
                    for gch in range(GCH):
                        ps = pep.tile([128, S * BL], F32, tag="pep")
                        for ech in range(ECH):
                            nc.tensor.matmul(
                                ps[:].rearrange("p (t b) -> p t b", b=BL),
                                wih_sb[:, (ech * GCH + gch) * 128:
                                       (ech * GCH + gch + 1) * 128],
                                xev[:, d, ech, :, :],
                                start=(ech == 0), stop=(ech == ECH - 1))
                        nc.scalar.activation(
                            zxv[:, :, d, gch, :],
                            ps[:].rearrange("p (t b) -> p t b", b=BL),
                            AF.Identity, bias=b_sb[:, gch:gch + 1])
                zyv = zyb[:].rearrange("p (t g b) -> p t g b", g=GCH, b=BL)
                ytv = yt_sb[:].rearrange("p (e t b) -> p e t b", e=ECH, b=BL)
                for gch in range(GCH):
                    ps = pep.tile([128, TD * BL], F32, tag="pep")
                    for ech in range(ECH):
                        nc.tensor.matmul(
                            ps[:].rearrange("p (t b) -> p t b", b=BL),
                            wihe_sb[:, (ech * GCH + gch) * 128:
                                    (ech * GCH + gch + 1) * 128],
                            ytv[:, ech, :, :],
                            start=(ech == 0), stop=(ech == ECH - 1))
                    nc.scalar.activation(
                        zyv[:, :, gch, :],
                        ps[:].rearrange("p (t b) -> p t b", b=BL),
                        AF.Identity, bias=bdec_sb[:, gch:gch + 1])

                # ---- encoder: both dirs fused per step ----
                whh_d = (whhf_sb, whhb_sb)
                for t in range(S):
                    # fwd prev state = slot t; bwd prev = slot 64-t
                    psl = (t, 64 - t)
                    zp = pz.tile([128, 128], F32, tag="z")
                    nc.tensor.matmul(zp[:], id128[:],
                                     zx[:, t * 128:(t + 1) * 128],
                                     start=True, stop=False,
                                     skip_group_check=True)
                    for d in range(2):
                        for gch in range(GCH):
                            for kch in range(HCH):
                                nc.tensor.matmul(
                                    zp[:, d * 64 + gch * 8:
                                       d * 64 + gch * 8 + 8],
                                    whh_d[d][:, (kch * GCH + gch) * 128:
                                             (kch * GCH + gch + 1) * 128],
                                    hv[:, d, kch, psl[d], :],
                                    start=False, stop=(kch == HCH - 1),
                                    skip_group_check=True)
                    # per-dir chains so fwd/bwd interleave across engines
                    for d in range(2):
                        s = work.tile([128, 64], F32, tag=f"s{d}")
                        nc.scalar.activation(s[:], zp[:, d * 64:d * 64 + 64],
                                             AF.Tanh)
                        sv = s[:].rearrange("p (g b) -> p g b", b=BL)
                        cd = c2[:, d * 16:d * 16 + 16]
                        A = work.tile([128, 16], F32, tag=f"A{d}")
                        Bt = work.tile([128, 16], F32, tag=f"B{d}")
                        # A = (s_f + 1) * C2 ;  B = (s_i + 1) * t_g
                        nc.vector.scalar_tensor_tensor(
                            A[:], sv[:, 2:4, :].rearrange("p c b -> p (c b)"),
                            1.0, cd, ALU.add, ALU.mult)
                        nc.vector.scalar_tensor_tensor(
                            Bt[:], sv[:, 0:2, :].rearrange("p c b -> p (c b)"),
                            1.0,
                            sv[:, 6:8, :].rearrange("p c b -> p (c b)"),
                            ALU.add, ALU.mult)
                        # C2' = 0.5*A + B
                        nc.vector.scalar_tensor_tensor(
                            cd, A[:], 0.5, Bt[:], ALU.mult, ALU.add)
                        tc_ = work.tile([128, 16], F32, tag=f"tc{d}")
                        nc.scalar.activation(tc_[:], cd, AF.Tanh, scale=0.5)
                        # H2 = (s_o+1)*tanh(c); fwd -> slot t+1, bwd -> 63-t
                        nc.vector.scalar_tensor_tensor(
                            hv[:, d, :, t + 1 if d == 0 else 63 - t, :],
                            sv[:, 4:6, :], 1.0,
                            tc_[:].rearrange("p (c b) -> p c b", b=BL),
                            ALU.add, ALU.mult)

                # ---- glue: encprojT, encWT, encW_cs, dec inits ----
                epw = encprojT[:].rearrange(
                    "p (c b s) -> p c s b", c=2, s=S)
                for mch in range(HCH):
                    ps = pep.tile([128, S * BL], F32, tag="pep")
                    for kc in range(4):
                        d, ch = kc // 2, kc % 2
                        # position p: fwd slot p+1, bwd slot p
                        rhs = (hv[:, 0, ch, 1:S + 1, :] if d == 0
                               else hv[:, 1, ch, 0:S, :])
                        nc.tensor.matmul(
                            ps[:], watt_sb[:, (kc * 2 + mch) * 128:
                                           (kc * 2 + mch + 1) * 128],
                            rhs, start=(kc == 0), stop=(kc == 3))
                    nc.scalar.activation(
                        epw[:, mch], ps[:].rearrange("p (s b) -> p s b", b=BL),
                        AF.Copy)
                for mch in range(HCH):
                    ps2 = pep.tile([128, S * BL], F32, tag="pep")
                    for kc in range(4):
                        d, ch = kc // 2, kc % 2
                        rhs = (hv[:, 0, ch, 1:S + 1, :] if d == 0
                               else hv[:, 1, ch, 0:S, :])
                        nc.tensor.matmul(
                            ps2[:], wca_sb[:, (kc * 2 + mch) * 128:
                                           (kc * 2 + mch + 1) * 128],
                            rhs, start=(kc == 0), stop=(kc == 3))
                    nc.scalar.activation(
                        encWT[:, mch * BL * S:(mch + 1) * BL * S],
                        ps2[:], AF.Copy)
                # transpose encWT -> encW_cs [(u,s), h] per (pair, mch)
                ewv = encWT[:].rearrange("p (c s b) -> p c s b", c=2, b=BL)
                for pair in range(4):
                    for mch in range(HCH):
                        pt = pep.tile([128, 128], BF16, tag="ptr")
                        for u in range(2):
                            nc.tensor.transpose(
                                pt[u * 64:(u + 1) * 64, :],
                                ewv[:, mch, :, 2 * pair + u], id128[:])
                        nc.vector.tensor_copy(
                            encW_cs[:, (pair * 2 + mch) * 128:
                                    (pair * 2 + mch + 1) * 128], pt[:])
                # decoder init: h0 = Wh @ [H2f;H2b], c0 = Wc @ [C2f;C2b]
                c2b = work.tile([128, 32], BF16, tag="c2b")
                nc.vector.tensor_copy(c2b[:], c2[:])
                hdec = work.tile([128, 16], BF16, tag="hdec")
                cdec = state.tile([128, 16], F32)
                pinit_h = psmall.tile([128, 16], F32, tag="po")
                pinit_c = psmall.tile([128, 16], F32, tag="po")
                for mch in range(HCH):
                    for kc in range(4):
                        d, ch = kc // 2, kc % 2
                        # final states: fwd = slot 64, bwd = slot 0
                        hfin = (hv[:, 0, ch, S, :] if d == 0
                                else hv[:, 1, ch, 0, :])
                        nc.tensor.matmul(
                            pinit_h[:, mch * 8:(mch + 1) * 8],
                            wh_sb[:, (kc * 2 + mch) * 128:
                                  (kc * 2 + mch + 1) * 128],
                            hfin,
                            start=(kc == 0), stop=(kc == 3))
                for mch in range(HCH):
                    for kc in range(4):
                        d, ch = kc // 2, kc % 2
                        nc.tensor.matmul(
                            pinit_c[:, mch * 8:(mch + 1) * 8],
                            wc_sb[:, (kc * 2 + mch) * 128:
                                  (kc * 2 + mch + 1) * 128],
                            c2b[:, (d * 2 + ch) * 8:(d * 2 + ch) * 8 + 8],
                            start=(kc == 0), stop=(kc == 3))
                nc.vector.tensor_copy(hdec[:], pinit_h[:])
                nc.vector.tensor_copy(cdec[:], pinit_c[:])
                gctx.close()

                # decoder-phase vocab streaming pools (mt 0..2 interleaved)
                if INTERLEAVE_VOCAB:
                    vwp = rctx.enter_context(tc.tile_pool(name="vwp", bufs=3))
                    vsc = rctx.enter_context(tc.tile_pool(name="vsc", bufs=2))
                    pvd = rctx.enter_context(
                        tc.tile_pool(name="pvd", bufs=1, space="PSUM"))
                NSL = [(0, 512), (512, 512), (1024, 512),
                       (1536, VBLK - 1536)]

                def vunit_mms(mt, blk):
                    wv_t = vwp.tile([128, 2 * VBLK], BF16, tag="wv")
                    for ch in range(HCH):
                        nc.sync.dma_start(
                            wv_t[:, ch * VBLK:(ch + 1) * VBLK],
                            wvt[:, ch * V + blk * VBLK:
                                ch * V + (blk + 1) * VBLK])
                    m = 128 if mt < 3 else 120
                    ntau = 16 if mt < 3 else 15
                    pvt = pvd.tile([128, VBLK], F32, tag="pv")
                    for (n0, nw) in NSL:
                        for ch in range(HCH):
                            lhs_ap = outsT[
                                :, ch * OST + (mt * 16 + 1) * 8:
                                ch * OST + (mt * 16 + 1 + ntau) * 8]
                            nc.tensor.matmul(
                                pvt[0:m, n0:n0 + nw],
                                lhs_ap,
                                wv_t[:, ch * VBLK + n0:ch * VBLK + n0 + nw],
                                start=(ch == 0), stop=(ch == HCH - 1))
                    return mt, blk, m, pvt

                def vunit_exp(u):
                    mt, blk, m, pvt = u
                    scr = vsc.tile([128, VBLK], BF16, tag="scr")
                    nc.scalar.activation(
                        scr[0:m, :], pvt[0:m, :], AF.Exp,
                        accum_out=se_parts[0:m, mt * NVB + blk:
                                           mt * NVB + blk + 1])

                # two units per step from t=16; mt0 ready after step 16 etc.
                pending_vus = []
                done_units = []

                # ---- decoder ----

                ovv = outsT[:].rearrange("p (c t b) -> p c t b", c=2, b=BL)
                for t in range(TD):
                    zp = pz.tile([128, 64], F32, tag="z")
                    nc.tensor.matmul(zp[:], id128[:],
                                     zyb[:, t * 64:(t + 1) * 64],
                                     start=True, stop=False,
                                     skip_group_check=True)
                    for si, (w_sb, rfn) in enumerate((
                            (wiho_sb, lambda k: ovv[:, k, t, :]),
                            (whhd_sb, lambda k: hdec[
                                :, k * 8:(k + 1) * 8]))):
                        for gch in range(GCH):
                            for kch in range(HCH):
                                nc.tensor.matmul(
                                    zp[:, gch * 8:(gch + 1) * 8],
                                    w_sb[:, (kch * GCH + gch) * 128:
                                         (kch * GCH + gch + 1) * 128],
                                    rfn(kch),
                                    start=False,
                                    stop=(si == 1 and kch == HCH - 1),
                                    skip_group_check=True)
                    s = work.tile([128, 64], F32, tag="s")
                    nc.scalar.activation(s[:], zp[:], AF.Tanh)
                    sv = s[:].rearrange("p (g b) -> p g b", b=BL)
                    A = work.tile([128, 16], F32, tag="A")
                    Bt = work.tile([128, 16], F32, tag="B")
                    nc.vector.scalar_tensor_tensor(
                        A[:], sv[:, 2:4, :].rearrange("p c b -> p (c b)"),
                        1.0, cdec[:], ALU.add, ALU.mult)
                    nc.vector.scalar_tensor_tensor(
                        Bt[:], sv[:, 0:2, :].rearrange("p c b -> p (c b)"),
                        1.0, sv[:, 6:8, :].rearrange("p c b -> p (c b)"),
                        ALU.add, ALU.mult)
                    nc.vector.scalar_tensor_tensor(
                        cdec[:], A[:], 0.5, Bt[:], ALU.mult, ALU.add)
                    tc_ = work.tile([128, 16], F32, tag="tc")
                    nc.scalar.activation(tc_[:], cdec[:], AF.Tanh, scale=0.5)
                    hnew = work.tile([128, 16], BF16, tag="hdec")
                    nc.vector.scalar_tensor_tensor(
                        hnew[:], sv[:, 4:6, :].rearrange("p c b -> p (c b)"),
                        1.0, tc_[:], ALU.add, ALU.mult)
                    hdec = hnew

                    # attention scores: out [(u,s), (pair,u')] psum
                    patt = psmall.tile([128, 16], F32, tag="patt")
                    pe = patt[:, 0:8]
                    for pair in range(4):
                        for mch in range(HCH):
                            lhs_ap = encprojT[
                                :, (mch * 4 + pair) * 128:
                                (mch * 4 + pair + 1) * 128]
                            nc.tensor.matmul(
                                pe[:, 2 * pair:2 * pair + 2],
                                lhs_ap,
                                hdec[:, mch * 8 + 2 * pair:
                                     mch * 8 + 2 * pair + 2],
                                start=(mch == 0), stop=(mch == 1))
                    # exp -> block-diag ablk (off-halves stay zero)
                    pev = pe.rearrange("p (j u) -> p j u", u=2)
                    abv = ablk[:].rearrange("p (j u) -> p j u", u=2)
                    nc.scalar.activation(abv[0:64, :, 0], pev[0:64, :, 0],
                                         AF.Exp)
                    nc.scalar.activation(abv[64:128, :, 1], pev[64:128, :, 1],
                                         AF.Exp)
                    # Z replicated to all 128 rows via ones matmul; 1/Z; mul
                    if pending_vus:
                        vunit_exp(pending_vus.pop(0))
                    zrep = patt[:, 8:16]
                    nc.tensor.matmul(zrep, ones_mat[:], ablk[:],
                                     start=True, stop=True)
                    rec = work.tile([128, 8], F32, tag="rec")
                    nc.vector.reciprocal(rec[:], zrep)
                    nc.vector.tensor_mul(ablk_n[:], ablk[:], rec[:])
                    # O_t = tanh(Wcomb_h' @ h2 + encW_cs^T @ alpha)
                    pot = psmall.tile([128, 16], F32, tag="po")
                    po = pot[:]
                    for mch in range(HCH):
                        for kch in range(HCH):
                            nc.tensor.matmul(
                                po[:, mch * 8:(mch + 1) * 8],
                                wch_sb[:, (kch * 2 + mch) * 128:
                                       (kch * 2 + mch + 1) * 128],
                                hdec[:, kch * 8:(kch + 1) * 8],
                                start=(kch == 0), stop=False,
                                skip_group_check=True)
                    for pair in range(4):
                        for mch in range(HCH):
                            nc.tensor.matmul(
                                po[:, mch * 8 + 2 * pair:
                                   mch * 8 + 2 * pair + 2],
                                encW_cs[:, (pair * 2 + mch) * 128:
                                        (pair * 2 + mch + 1) * 128],
                                ablk_n[:, 2 * pair:2 * pair + 2],
                                start=False, stop=True,
                                skip_group_check=True)
                    nc.scalar.activation(ovv[:, 0, t + 1, :],
                                         po[:, 0:8], AF.Tanh)
                    nc.scalar.activation(ovv[:, 1, t + 1, :],
                                         po[:, 8:16], AF.Tanh)
                    if pending_vus:
                        vunit_exp(pending_vus.pop(0))
                    if INTERLEAVE_VOCAB and 16 <= t:
                        for _ in range(2):
                            ui = len(done_units)
                            mt, blk = ui // NVB, ui % NVB
                            if mt > 2 or mt * 16 + 16 > t:
                                break
                            pending_vus.append(vunit_mms(mt, blk))
                            done_units.append((mt, blk))

                for u in pending_vus:
                    vunit_exp(u)

            # ---- vocab projection + exp-sum (stream Wvocab from HBM) ----
            with ExitStack() as vctx:
                vwp = vctx.enter_context(tc.tile_pool(name="vwp2", bufs=3))
                vsc = vctx.enter_context(tc.tile_pool(name="vsc2", bufs=2))
                pv = vctx.enter_context(
                    tc.tile_pool(name="pv", bufs=2, space="PSUM"))
                # gold logits: dot(O_t, Wvocab[gold_t]) via ones-matmul
                ovf = outsT[:].rearrange("p (c t b) -> p c t b", c=2, b=BL)
                ov = ovf[:, :, 1:, :]
                wgv = wgt_sb[:].rearrange("p (c t b) -> p c t b", c=2, b=BL)
                tgv = tmp_gd[:].rearrange("p (c t b) -> p c t b", c=2, b=BL)
                nc.vector.tensor_mul(tgv, ov, wgv)
                pgd = pv.tile([1, 1024], F32, tag="pv")
                nc.tensor.matmul(pgd[0:1, 0:NR], ones_col[:],
                                 tmp_gd[:, 0:NR], start=True, stop=True)
                nc.tensor.matmul(pgd[0:1, 512:512 + NR], ones_col[:],
                                 tmp_gd[:, NR:2 * NR], start=True, stop=True)
                nc.scalar.activation(gd_sb[:, 0:NR], pgd[0:1, 0:NR], AF.Copy)
                nc.scalar.activation(gd_sb[:, 512:512 + NR],
                                     pgd[0:1, 512:512 + NR], AF.Copy)
                all_units = [(mt, blk) for mt in range(4)
                             for blk in range(NVB)]
                for blk in range(NVB):
                    todo = [(mt, b) for (mt, b) in all_units
                            if b == blk and (mt, b) not in done_units]
                    if not todo:
                        continue
                    wv_t = vwp.tile([128, 2 * VBLK], BF16, tag="wv")
                    for ch in range(HCH):
                        nc.sync.dma_start(
                            wv_t[:, ch * VBLK:(ch + 1) * VBLK],
                            wvt[:, ch * V + blk * VBLK:
                                ch * V + (blk + 1) * VBLK])
                    for mt, _b in todo:
                        m = 128 if mt < 3 else 120
                        ntau = 16 if mt < 3 else 15
                        pvt = pv.tile([128, VBLK], F32, tag="pv")
                        nsl = [(0, 512), (512, 512), (1024, 512),
                               (1536, VBLK - 1536)]
                        for (n0, nw) in nsl:
                            for ch in range(HCH):
                                lhs_ap = outsT[
                                    :, ch * OST + (mt * 16 + 1) * 8:
                                    ch * OST + (mt * 16 + 1 + ntau) * 8]
                                nc.tensor.matmul(
                                    pvt[0:m, n0:n0 + nw],
                                    lhs_ap,
                                    wv_t[:, ch * VBLK + n0:
                                         ch * VBLK + n0 + nw],
                                    start=(ch == 0), stop=(ch == HCH - 1))
                        scr = vsc.tile([128, VBLK], BF16, tag="scr")
                        nc.scalar.activation(
                            scr[0:m, :], pvt[0:m, :], AF.Exp,
                            accum_out=se_parts[0:m, mt * NVB + blk:
                                               mt * NVB + blk + 1])
                sev = se_parts[:].rearrange("p (mt k) -> p mt k", k=NVB)
                for mt in range(4):
                    nc.vector.tensor_reduce(
                        lse_sb[:, mt:mt + 1], sev[:, mt, :],
                        axis=mybir.AxisListType.X, op=mybir.AluOpType.add)
                lse2 = state.tile([128, 4], F32)
                nc.scalar.activation(lse2[:], lse_sb[:], AF.Ln)
                nc.sync.dma_start(out_lse[:], lse2[:])
                nc.sync.dma_start(out_gd[:], gd_sb[:])

    nc.compile()
    return nc


# ---------------- host-side packing ----------------

# torch gate order (i, f, g, o) -> new order (i, f, o, g)
_PERM = np.concatenate([np.arange(0, 512), np.arange(768, 1024),
                        np.arange(512, 768)])
# rows 0:768 (i,f,o) get the tanh-half 0.5 prescale
_RS = np.concatenate([np.full(768, 0.5, np.float32),
                      np.ones(256, np.float32)])[:, None]


def _pack_lhsT(wt, kchs, mchs):
    """wt: (K, M) = W.T -> (128, kchs*mchs*128), col=(kch*mchs+mch)*128+m."""
    tiles = [wt[k * 128:(k + 1) * 128, m * 128:(m + 1) * 128]
             for k in range(kchs) for m in range(mchs)]
    return np.ascontiguousarray(np.concatenate(tiles, axis=1)).astype(bf16)


def _pack_xT(x):
    """x: (rows, 256) -> (128, 2*rows), col = ech*rows + r."""
    a = np.ascontiguousarray(x.T)
    return np.ascontiguousarray(
        np.concatenate([a[:128], a[128:]], axis=1)).astype(bf16)


def _pack_bias(b):
    return np.ascontiguousarray(b.reshape(GCH, 128).T).astype(np.float32)


_NC_CACHE = {}
_RUN_KWARGS = {}
_LAST_RESULTS = None
_LAST_INMAPS = None


def _get_program():
    if "nc" not in _NC_CACHE:
        _NC_CACHE["nc"] = build_program()
    return _NC_CACHE["nc"]


def kernel(source_padded, target_padded, src_emb, tgt_emb,
           enc_Wih_f, enc_Whh_f, enc_b_f, enc_Wih_b, enc_Whh_b, enc_b_b,
           dec_Wih, dec_Whh, dec_b, Wh, Wc, Watt, Wcomb, Wvocab):
    source_padded = np.asarray(source_padded)
    target_padded = np.asarray(target_padded)
    src_emb = np.asarray(src_emb, np.float32)
    tgt_emb = np.asarray(tgt_emb, np.float32)
    Wvocab = np.asarray(Wvocab, np.float32)
    nc = _get_program()

    def gatefix(W):
        return _RS * np.asarray(W, np.float32)[_PERM]

    shared = {
        "wih_f": _pack_lhsT(gatefix(enc_Wih_f).T, ECH, GCH),
        "wih_b": _pack_lhsT(gatefix(enc_Wih_b).T, ECH, GCH),
        "whh_f": _pack_lhsT((0.5 * gatefix(enc_Whh_f)).T, HCH, GCH),
        "whh_b": _pack_lhsT((0.5 * gatefix(enc_Whh_b)).T, HCH, GCH),
        "benc_f": _pack_bias(_RS[:, 0] * np.asarray(enc_b_f, np.float32)[_PERM]),
        "benc_b": _pack_bias(_RS[:, 0] * np.asarray(enc_b_b, np.float32)[_PERM]),
        "wihe": _pack_lhsT(gatefix(np.asarray(dec_Wih)[:, :E]).T, ECH, GCH),
        "wiho": _pack_lhsT(gatefix(np.asarray(dec_Wih)[:, E:]).T, HCH, GCH),
        "whhd": _pack_lhsT((0.5 * gatefix(dec_Whh)).T, HCH, GCH),
        "bdec": _pack_bias(_RS[:, 0] * np.asarray(dec_b, np.float32)[_PERM]),
        "wcomb_a": _pack_lhsT((0.5 * np.asarray(Wcomb, np.float32)[:, :2 * H]).T,
                              4, HCH),
        "wcomb_h": _pack_lhsT((0.5 * np.asarray(Wcomb, np.float32)[:, 2 * H:]).T,
                              HCH, HCH),
        "wh_l": _pack_lhsT(np.asarray(Wh, np.float32).T, 4, HCH),
        "wc_l": _pack_lhsT(np.asarray(Wc, np.float32).T, 4, HCH),
        "watt_l": _pack_lhsT((0.25 * np.asarray(Watt, np.float32)).T, 4, HCH),
        "wvt": _pack_xT(Wvocab),
    }

    in_maps = []
    for c in range(NCORES):
        bs = slice(BL * c, BL * (c + 1))
        src = source_padded[:, bs]
        tgt = target_padded[:, bs]
        X = src_emb[src]                      # (S, 8, E)
        Y = tgt_emb[tgt[:-1]]                 # (TD, 8, E)
        wg = Wvocab[tgt[1:].reshape(-1)]      # (504, 256)
        m = dict(shared)
        # xe: (dir, ech, t, b): dir0 = fwd x_t, dir1 = bwd x_{S-1-t}
        xe = np.stack([X, X[::-1]], axis=0)   # (2, S, 8, E)
        a = xe.reshape(2 * S * BL, E).T       # (E, 2*S*8)
        a = a.reshape(2, 128, 2, S, BL)       # (ech, p, d, t, b)
        a = a.transpose(1, 2, 0, 3, 4)        # (p, d, ech, t, b)
        m["xe_t"] = np.ascontiguousarray(
            a.reshape(128, S * 2 * ECH * BL)).astype(bf16)
        m["yt"] = _pack_xT(Y.reshape(TD * BL, E))
        m["wgt"] = _pack_xT(wg)
        in_maps.append(m)

    r = run_bass_kernel_spmd(nc, in_maps, list(range(NCORES)),
                             **_RUN_KWARGS)
    global _LAST_RESULTS, _LAST_INMAPS
    _LAST_RESULTS = r
    _LAST_INMAPS = in_maps

    out = np.zeros(B, np.float32)
    for c in range(NCORES):
        lse = r.results[c]["out_lse"]
        gd = r.results[c]["out_gd"][0]
        lse_flat = lse.T.reshape(-1)[:NR]
        gold_logit = gd[:NR] + gd[512:512 + NR]
        p_gold = (gold_logit - lse_flat).reshape(TD, BL)
        mask = (target_padded[1:, BL * c:BL * (c + 1)] != 0)
        out[BL * c:BL * (c + 1)] = (p_gold * mask).sum(axis=0)
    return out
